# revision 35
# baseline (speedup 1.0000x reference)
"""Trainium2 Bass kernel: MultiHeadDepthwiseSelfAttention.

Full inputs -> data-parallel over batch across 8 NeuronCores -> full output.

Math (reference):
    q = x*wq + bq; k = x*wk + bk; v = x*wv + bv      (per-channel depthwise)
    att = softmax(q @ k^T / sqrt(F))  per head        (H=12, d=64)
    out = (att @ v) * wo + bo

Kernel strategy (per core, 2 batches):
  - Softmax over k is invariant to per-q logit shifts, so
    q_i.k_j = (wq wk x_i + wk bq).x_j + c_i  ->  Q^T = x^T*w2 + wb (one DVE
    pass), K^T = raw x^T (plain copy).  All weight folds happen on host:
    w2 = wq*wk, wb = wk*bq, veff = wv*wo, beff = bv*wo + bo.
  - x is converted fp32->bf16 on GpSimd into an interleaved layout with a
    ones-column per head ([64 ch | 1] x 12 = 780 cols).  PV matmuls read these
    raw bf16 tiles directly as stationary operands; the ones column makes the
    PV matmul itself accumulate softmax's normalizer Z (row d of O^T).
  - S^T[k,q] per head via row-tiled matmul pairs (d=64 contraction at row
    groups 0/64 -> the two heads of a pair run concurrently in the PE array).
  - exp() on ACT (no max-subtraction needed: logits are O(1)), bf16 out.
  - O^T is veff-scaled during the PSUM->SBUF drain copy (per-partition scalar
    in transposed space; Z row scales by 1), PE-transposed back to natural
    layout, then normalized with 1/Z and beff added in one DVE op.
  - Everything 2-byte where PE/DVE benefit: transposes at 1 cycle/row, DVE
    copies in 2x mode.
"""

import math
import os
import sys

for _p in ("/opt/trn_rl_repo", "/root/.axon_site/_ro/trn_rl_repo"):
    if os.path.isdir(_p) and _p not in sys.path:
        sys.path.insert(0, _p)

import numpy as np

import concourse.bacc as bacc
import concourse.mybir as mybir
from concourse.tile import TileContext
from concourse.masks import make_identity
from concourse.bass_utils import run_bass_kernel_spmd

FP32 = mybir.dt.float32
BF16 = mybir.dt.bfloat16
AF = mybir.ActivationFunctionType
ALU = mybir.AluOpType

P = 128
N_CORES = 8
B, N, F, H = 16, 1024, 768, 12


def build(BPC=2, N=N, F=F, H=H, reps=1, loop_reps=None, stages=4,
          budget=1, dkc=4, ptb=3, otb=1, qsrc="pst", fastpro=True):
    d = F // H            # head dim (64)
    dO = d + 1            # cols per head incl. ones column
    NT = N // P           # n-tiles (= k-chunks)
    CT = H // 2           # head pairs (= 128-channel chunks)
    QB = min(512, N)      # q block (moving-dim) size
    QC = N // QB          # q blocks
    TB = QB // P          # natural q-subtiles per q block
    XW = H * dO           # interleaved x width (780)
    scale = 1.0 / math.sqrt(F)
    assert P % d == 0 and CT * P == F

    nc = bacc.Bacc("TRN2", target_bir_lowering=False, debug=False,
                   num_devices=N_CORES)
    x = nc.declare_dram_parameter("x", [BPC, N, F], FP32, isOutput=False)
    w2c = nc.declare_dram_parameter("w2c", [P, CT], FP32, isOutput=False)
    wbc = nc.declare_dram_parameter("wbc", [P, CT], FP32, isOutput=False)
    vcol = nc.declare_dram_parameter("vcol", [dO, H], FP32, isOutput=False)
    beff = nc.declare_dram_parameter("beff", [F], FP32, isOutput=False)
    out = nc.declare_dram_parameter("out", [BPC, N, F], FP32, isOutput=True)

    with TileContext(nc) as tc:
        with (
            tc.tile_pool(name="const", bufs=1) as cpool,
            tc.tile_pool(name="xfp", bufs=2) as xfpool,
            tc.tile_pool(name="xbp", bufs=2) as xbpool,
            tc.tile_pool(name="xtp", bufs=2) as xtpool,
            tc.tile_pool(name="qp", bufs=1) as qpool,
            tc.tile_pool(name="kp", bufs=1) as kpool,
            tc.tile_pool(name="op", bufs=1) as opool,
            tc.tile_pool(name="ptp", bufs=ptb) as ptpool,
            tc.tile_pool(name="otp", bufs=otb) as otpool,
            tc.tile_pool(name="rzp", bufs=2) as rzpool,
            tc.tile_pool(name="ps_s", bufs=2, space="PSUM") as ps_s,
            tc.tile_pool(name="ps_o", bufs=2, space="PSUM") as ps_o,
            tc.tile_pool(name="ps_n", bufs=1, space="PSUM") as ps_n,
        ):
            ident = cpool.tile([P, P], BF16)
            make_identity(nc, ident[:])
            w2_c = cpool.tile([P, CT], FP32)
            wb_c = cpool.tile([P, CT], FP32)
            v_col = cpool.tile([dO, H], FP32)
            beff_b = cpool.tile([P, F], FP32)

            def emit_const_loads():
                nc.sync.dma_start(out=w2_c[:], in_=w2c[:, :])
                nc.sync.dma_start(out=wb_c[:], in_=wbc[:, :])
                nc.sync.dma_start(out=v_col[:], in_=vcol[:, :])
                nc.sync.dma_start(out=beff_b[:],
                                  in_=beff[None, :].broadcast_to([P, F]))

            def alloc_x(_bi):
                xfs = [xfpool.tile([P, F], FP32, tag=f"xf{i}", name=f"xf{i}")
                       for i in range(NT)]
                xbs = [xbpool.tile([P, XW], BF16, tag=f"xb{i}", name=f"xb{i}")
                       for i in range(NT)]
                xts = [xtpool.tile([P, F], BF16, tag=f"xr{i}", name=f"xr{i}")
                       for i in range(NT)]
                return xfs, xbs, xts

            def alloc_qk(_bi):
                qts = [qpool.tile([P, N], BF16, tag=f"qt{c}", name=f"qt{c}")
                       for c in range(CT)]
                kts = [kpool.tile([P, N], BF16, tag=f"kt{c}", name=f"kt{c}")
                       for c in range(CT)]
                return qts, kts

            def emit_conv_xts(xfs, xts, i):
                # fp32 -> bf16 contiguous copy (transpose source)
                nc.gpsimd.tensor_copy(out=xts[i][:], in_=xfs[i][:])

            def emit_conv_xbf(xfs, xbs, i):
                # fp32 -> bf16 per-head interleave + ones column (PV source)
                x3 = xfs[i].rearrange("p (h e) -> p h e", e=d)
                b3 = xbs[i].rearrange("p (h e) -> p h e", e=dO)
                nc.gpsimd.tensor_copy(out=b3[:, :, 0:d], in_=x3[:])
                nc.gpsimd.memset(xbs[i][:, d::dO], 1.0)

            def emit_chunk(xts, qts, kts, c):
                # transpose chunk c (heads 2c,2c+1); K^T = raw copy (the only
                # PSUM read), Q^T scaled from SBUF K^T
                pst = ps_n.tile([P, N], BF16, tag="pst", name="pst")
                for j in range(NT):
                    nc.tensor.transpose(pst[:, j * P:(j + 1) * P],
                                        xts[j][:, c * P:(c + 1) * P],
                                        ident[:])
                nc.vector.tensor_copy(out=kts[c][:], in_=pst[:])
                nc.vector.tensor_scalar(qts[c][:],
                                        kts[c][:] if qsrc == "kts" else pst[:],
                                        w2_c[:, c:c + 1], wb_c[:, c:c + 1],
                                        op0=ALU.mult, op1=ALU.add)

            def emit_program(batches):
                NB = len(batches)
                phases = [(bi, c, qc) for bi in range(NB)
                          for c in range(CT) for qc in range(QC)]
                NPH = len(phases)
                pidx = {ph: i for i, ph in enumerate(phases)}

                xb_of, qk_of, outs_of = {}, {}, {}

                def get_outs(bi):
                    if bi not in outs_of:
                        outs_of[bi] = {
                            i: opool.tile([P, F], FP32, tag=f"on{i}",
                                          name=f"on{i}") for i in range(NT)}
                    return outs_of[bi]

                # work items: (earliest, deadline, fn); deadline=None -> any
                items = []

                def add_batch_items(bi):
                    xfs, xbs, xts = alloc_x(bi)
                    xb_of[bi] = xbs
                    qk_of[bi] = alloc_qk(bi)
                    qts, kts = qk_of[bi]
                    first = pidx[(bi, 0, 0)]
                    if bi == 0:
                        ear_load = 0
                    else:
                        ear_load = pidx[(bi - 1, 0, 0)]

                    def xload(bi=bi, xfs=xfs):
                        if fastpro:
                            # chunk-0 columns first: unblocks the first
                            # transposes (and first exp) several us earlier
                            for i in range(NT):
                                nc.sync.dma_start(
                                    out=xfs[i][:, 0:P],
                                    in_=x[batches[bi],
                                         i * P:(i + 1) * P, 0:P])
                            for i in range(NT):
                                nc.sync.dma_start(
                                    out=xfs[i][:, P:],
                                    in_=x[batches[bi], i * P:(i + 1) * P, P:])
                        else:
                            for i in range(NT):
                                nc.sync.dma_start(
                                    out=xfs[i][:],
                                    in_=x[batches[bi], i * P:(i + 1) * P, :])
                    if bi == 0:
                        def first_loads():
                            emit_const_loads()
                            xload()
                        items.append((0, 0, first_loads))
                    else:
                        items.append((ear_load, first, xload))
                    # xts (transpose-source) conversions first, then xbf
                    if fastpro:
                        def conv_c0(xfs=xfs, xts=xts):
                            for i in range(NT):
                                nc.gpsimd.tensor_copy(out=xts[i][:, 0:P],
                                                      in_=xfs[i][:, 0:P])
                        items.append((ear_load, first, conv_c0))

                        def conv_rest(g, xfs=xfs, xts=xts):
                            for i in range(4 * g, 4 * g + 4):
                                nc.gpsimd.tensor_copy(out=xts[i][:, P:],
                                                      in_=xfs[i][:, P:])
                        items.append((ear_load, first, lambda: conv_rest(0)))
                        items.append((ear_load, first, lambda: conv_rest(1)))
                    else:
                        for g in range(2):
                            items.append((ear_load, first,
                                          lambda g=g, xfs=xfs, xts=xts:
                                          [emit_conv_xts(xfs, xts, i)
                                           for i in range(4 * g, 4 * g + 4)]))
                    for g in range(2):
                        items.append((ear_load, first,
                                      lambda g=g, xfs=xfs, xbs=xbs:
                                      [emit_conv_xbf(xfs, xbs, i)
                                       for i in range(4 * g, 4 * g + 4)]))
                    for c in range(CT):
                        # chunk c of batch bi: after cur batch stops reading
                        # chunk c (pair c+1 of batch bi-1), before (bi, c, 0)
                        if bi == 0:
                            ear = 0
                        elif c + 1 < CT:
                            ear = pidx[(bi - 1, c + 1, 0)]
                        else:
                            ear = pidx[(bi, 0, 0)]
                        items.append((ear, pidx[(bi, c, 0)],
                                      lambda c=c, xts=xts, qts=qts, kts=kts:
                                      emit_chunk(xts, qts, kts, c)))

                for bi in range(NB):
                    add_batch_items(bi)
                items.sort(key=lambda it: (it[0], it[1] if it[1] is not None
                                           else NPH))

                def flush(i, forced_deadline, budget=budget):
                    # emit all items whose deadline <= forced_deadline, plus
                    # up to `budget` items whose earliest <= i
                    rest = []
                    n = 0
                    for it in items:
                        ear, dl, fn = it
                        if dl is not None and dl <= forced_deadline:
                            fn()
                        elif ear <= i and n < budget:
                            fn()
                            n += 1
                        else:
                            rest.append(it)
                    items[:] = rest

                # pipeline state
                po2_of, pts_of, pending = {}, {}, []

                def emit_s_exp(i, kc):
                    bi, c, qc = phases[i]
                    qts, kts = qk_of[bi]
                    ps = ps_s.tile([P, 2 * QB], FP32, tag="ps", name="ps")
                    for e in range(2):
                        nc.tensor.matmul(
                            ps[:, e * QB:(e + 1) * QB],
                            lhsT=kts[c][e * d:(e + 1) * d, kc * P:(kc + 1) * P],
                            rhs=qts[c][e * d:(e + 1) * d, qc * QB:(qc + 1) * QB],
                            start=True, stop=True)
                    pt = ptpool.tile([P, 2 * QB], BF16, tag="pt", name="pt")
                    nc.scalar.activation(pt[:], ps[:], AF.Exp, scale=scale)
                    pts_of[(i, kc)] = pt

                def emit_pv(i, kc):
                    if stages < 3:
                        return
                    bi, c, qc = phases[i]
                    xbs = xb_of[bi]
                    h0 = 2 * c
                    if i not in po2_of:
                        po2_of[i] = [ps_o.tile([dO, QB], FP32, tag="po",
                                               name=f"po{e}") for e in range(2)]
                    pt = pts_of.pop((i, kc))
                    for e in range(2):
                        nc.tensor.matmul(
                            po2_of[i][e][:],
                            lhsT=xbs[kc][:, (h0 + e) * dO:(h0 + e + 1) * dO],
                            rhs=pt[:, e * QB:(e + 1) * QB],
                            start=(kc == 0), stop=(kc == NT - 1))

                def emit_drain(i):
                    if stages < 4:
                        return
                    bi, c, qc = phases[i]
                    last_pair = (c == CT - 1)
                    h0 = 2 * c
                    outs = get_outs(bi)
                    po2 = po2_of.pop(i)
                    dP = dO + 1  # pad per-t stride for 4B PSUM alignment
                    EW = TB * dP  # per-e width in merged pn
                    ots = []
                    for e in range(2):
                        ot = otpool.tile([dO, QB], BF16, tag=f"ot{e}",
                                         name=f"ot{e}")
                        # veff applied in the PSUM drain copy (per-partition
                        # in O^T space; the Z row's scale is 1)
                        nc.vector.tensor_scalar(
                            ot[:], po2[e][:], v_col[:, h0 + e:h0 + e + 1],
                            0.0, op0=ALU.mult, op1=ALU.add)
                        ots.append(ot)

                    box = {}

                    def finish_tr():
                        pn = ps_n.tile([P, 2 * EW], BF16, tag="pn", name="pn")
                        for e in range(2):
                            for t in range(TB):
                                nc.tensor.transpose(
                                    pn[:, e * EW + t * dP:e * EW + t * dP + dO],
                                    ots[e][:, t * P:(t + 1) * P],
                                    ident[0:dO, 0:dO])
                        rz = rzpool.tile([P, 2 * TB], FP32, tag="rz",
                                         name="rz")
                        nc.vector.reciprocal(rz[:], pn[:, d::dP])
                        box["pn"], box["rz"] = pn, rz

                    def finish_st():
                        pn, rz = box["pn"], box["rz"]
                        for e in range(2):
                            for t in range(TB):
                                qsub = qc * TB + t
                                cl = (h0 + e) * d
                                nc.vector.scalar_tensor_tensor(
                                    out=outs[qsub][:, cl:cl + d],
                                    in0=pn[:, e * EW + t * dP:
                                           e * EW + t * dP + d],
                                    scalar=rz[:, e * TB + t:e * TB + t + 1],
                                    in1=beff_b[:, cl:cl + d],
                                    op0=ALU.mult, op1=ALU.add)
                        if last_pair:
                            for t in range(TB):
                                qsub = qc * TB + t
                                nc.sync.dma_start(
                                    out=out[batches[bi],
                                            qsub * P:(qsub + 1) * P, :],
                                    in_=outs[qsub][:])
                    pending.extend([finish_tr, finish_st])

                def flush_pending(n=99):
                    for _ in range(n):
                        if pending:
                            pending.pop(0)()

                # prologue: phase 0 prefetch
                flush(0, 0, budget=0)
                emit_s_exp(0, 0)
                for i in range(NPH):
                    bi, c, qc = phases[i]
                    for kc in range(1, NT):
                        emit_s_exp(i, kc)
                        if kc < NT - 1:
                            emit_pv(i, kc - 1)
                        if kc in (dkc, dkc + 3):
                            flush_pending(1)
                    if i + 1 < NPH:
                        flush(i, i + 1)
                        emit_s_exp(i + 1, 0)
                    emit_pv(i, NT - 2)
                    emit_pv(i, NT - 1)
                    emit_drain(i)
                    # end of batch: flush the last drain, drop out-tile refs
                    if stages >= 4 and (i + 1 == NPH or phases[i + 1][0] != bi):
                        flush_pending()
                        outs_of.pop(bi)
                flush(NPH, NPH, budget=99)

            if loop_reps is None:
                emit_program([bb for _ in range(reps) for bb in range(BPC)])
            else:
                with tc.For_i(0, loop_reps, 1):
                    emit_program(list(range(BPC)))
    nc.compile()
    return nc


_built = {}


def _get_nc(BPC):
    if BPC not in _built:
        _built[BPC] = build(BPC=BPC)
    return _built[BPC]


def fold_weights(wq, bq, wk, bk, wv, bv, wo, bo, F=F, H=H):
    d = F // H
    CT = H // 2
    w2 = (wq * wk).astype(np.float32)
    wb = (wk * bq).astype(np.float32)
    veff = (wv * wo).astype(np.float32)
    beff = (bv * wo + bo).astype(np.float32)
    w2c = np.ascontiguousarray(w2.reshape(CT, P).T)
    wbc = np.ascontiguousarray(wb.reshape(CT, P).T)
    vcol = np.concatenate(
        [veff.reshape(H, d).T, np.ones((1, H), np.float32)], axis=0)
    return {"w2c": w2c, "wbc": wbc,
            "vcol": np.ascontiguousarray(vcol), "beff": beff}


def make_in_maps(inputs):
    x = np.ascontiguousarray(np.asarray(inputs["x"], dtype=np.float32))
    folds = fold_weights(*(np.asarray(inputs[k], dtype=np.float32)
                           for k in ("wq", "bq", "wk", "bk",
                                     "wv", "bv", "wo", "bo")))
    Bx = x.shape[0]
    BPC = Bx // N_CORES
    assert BPC * N_CORES == Bx, (Bx, N_CORES)
    return [{"x": x[i * BPC:(i + 1) * BPC], **folds}
            for i in range(N_CORES)], BPC


def kernel(x, wq, bq, wk, bk, wv, bv, wo, bo):
    in_maps, BPC = make_in_maps(dict(x=x, wq=wq, bq=bq, wk=wk, bk=bk,
                                     wv=wv, bv=bv, wo=wo, bo=bo))
    nc = _get_nc(BPC)
    res = run_bass_kernel_spmd(nc, in_maps, list(range(N_CORES)))
    return np.concatenate([r["out"] for r in res.results], axis=0)


if __name__ == "__main__":
    rng = np.random.default_rng(1)
    inputs = {
        "x": rng.standard_normal((B, N, F), dtype=np.float32),
        "wq": rng.standard_normal((F,), dtype=np.float32),
        "bq": np.zeros(F, np.float32),
        "wk": rng.standard_normal((F,), dtype=np.float32),
        "bk": np.zeros(F, np.float32),
        "wv": rng.standard_normal((F,), dtype=np.float32),
        "bv": np.zeros(F, np.float32),
        "wo": rng.standard_normal((F,), dtype=np.float32),
        "bo": np.zeros(F, np.float32),
    }
    o = kernel(**inputs)
    print("out", o.shape, o.dtype)
